# revision 4
# baseline (speedup 1.0000x reference)
"""Trainium2 Bass kernel for nn_CLF_block (channel-attention block).

Reference computation (per batch item b, with x = concat([a,b], ch) in [256, N],
N = H*W = 16384):
    z  = w1 x + b1 1^T
    q  = w2 z + b2 1^T ;  k = w3 z + b3 1^T ;  v = w4 z + b4 1^T
    qk = q k^T ; attn = softmax(qk, -1) ; out = attn v

Algebraic restructuring (verified vs reference, max-rel ~1e-4):
    Gx = x x^T                [256,256]   (one pass over x)
    sx = x 1                  [256]
    u  = w1 sx ; s = u + N b1
    G  = w1 Gx w1^T + u b1^T + b1 u^T + N b1 b1^T        (= z z^T)
    qk = w2 G w3^T + (w2 s) b3^T + b2 (w3 s)^T + N b2 b3^T
    attn = softmax(qk)
    M  = attn w4 ; W = M w1 ; c0 = M b1 + attn b4
    out = W x + c0 1^T        (second pass over x)

So only two O(256*256*N) passes over x touch HBM-sized data; everything else is
256x256 algebra. HBM traffic per core = 16 MiB in + 16 MiB out (x stays in SBUF
between the passes) -> memory-bound.

Sharding: data-parallel over batch, one batch item per NeuronCore (B=8, 8 cores).
"""

import sys

if "/opt/trn_rl_repo" not in sys.path:
    sys.path.insert(0, "/opt/trn_rl_repo")

from contextlib import ExitStack

import numpy as np

import concourse.bass as bass
import concourse.mybir as mybir
import concourse.tile as tile
from concourse import bacc
from concourse.bass_utils import run_bass_kernel_spmd

F32 = mybir.dt.float32
P = 128          # partitions / channel block
C = 256          # channels
NPIX = 128 * 128  # spatial positions per batch item
NPIECE = 16       # resident x pieces per input half
PIECE = NPIX // NPIECE   # 1024 cols per piece
NCHUNK = NPIX // P       # 128 gram chunks
OUTW = 2048       # output staging tile width
NT = 512          # matmul moving-operand width for pass 2


def _emit(nc, tc, ctx, d_in, d_out):
    """Emit the Tile program for one core (one batch item)."""
    xa, xb, wcat, ident = d_in["xa"], d_in["xb"], d_in["wcat"], d_in["ident"]
    brows, bcols = d_in["brows"], d_in["bcols"]
    out_d = d_out["out"]

    const = ctx.enter_context(tc.tile_pool(name="const", bufs=1))
    xpool = ctx.enter_context(tc.tile_pool(name="xpool", bufs=1))

    # --- constants -------------------------------------------------------
    w_sb = []
    for k in range(2):
        wt = const.tile([P, 5 * C], F32, name=f"w_sb{k}", tag=f"w_sb{k}")
        nc.sync.dma_start(out=wt, in_=wcat[k * P:(k + 1) * P, :])
        w_sb.append(wt)
    w1t = [w_sb[k][:, 0 * C:1 * C] for k in range(2)]   # w1^T  [cin, o]
    w1r = [w_sb[k][:, 1 * C:2 * C] for k in range(2)]   # w1    [o, cin]
    w2t = [w_sb[k][:, 2 * C:3 * C] for k in range(2)]   # w2^T
    w3t = [w_sb[k][:, 3 * C:4 * C] for k in range(2)]   # w3^T
    w4r = [w_sb[k][:, 4 * C:5 * C] for k in range(2)]   # w4    [d', d]

    rows = []
    for r in range(5):
        rt = const.tile([1, C], F32, name=f"brow{r}", tag=f"brow{r}")
        nc.sync.dma_start(out=rt, in_=brows[r:r + 1, :])
        rows.append(rt)
    b1_row, nb1_row, b2_row, b3_row, nb3_row = rows

    bc_sb = []
    for k in range(2):
        bt = const.tile([P, 3], F32, name=f"bcol{k}", tag=f"bcol{k}")
        nc.sync.dma_start(out=bt, in_=bcols[k * P:(k + 1) * P, :])
        bc_sb.append(bt)
    b1_col = [bc_sb[k][:, 0:1] for k in range(2)]
    nb1_col = [bc_sb[k][:, 1:2] for k in range(2)]
    b4_col = [bc_sb[k][:, 2:3] for k in range(2)]

    ident_sb = const.tile([P, P], F32, name="ident_sb", tag="ident_sb")
    nc.sync.dma_start(out=ident_sb, in_=ident[:, :])

    # --- resident input x (two channel halves, 16 pieces each) -----------
    xs = [[], []]
    for c, src in ((0, xa), (1, xb)):
        for i in range(NPIECE):
            xt = xpool.tile([P, PIECE], F32, name=f"x{c}_{i}", tag=f"x{c}_{i}")
            nc.sync.dma_start(out=xt, in_=src[:, i * PIECE:(i + 1) * PIECE])
            xs[c].append(xt)

    # --- pass 1: Gx = x x^T (augmented with row-sum column) --------------
    gx_sb = [
        const.tile([P, C + 1], F32, name=f"gx_sb{b}", tag=f"gx_sb{b}")
        for b in range(2)
    ]
    with tc.tile_pool(name="gx_ps", bufs=1, space="PSUM") as gxp, \
         tc.tile_pool(name="tr_ps", bufs=4, space="PSUM") as trp, \
         tc.tile_pool(name="xt_sb", bufs=3) as xtp:
        gx_ps = [
            gxp.tile([P, C + 1], F32, name=f"gx_ps{b}", tag=f"gx_ps{b}")
            for b in range(2)
        ]
        chunks_per_piece = PIECE // P
        for i in range(NPIECE):
            for j in range(chunks_per_piece):
                ch = i * chunks_per_piece + j
                xt = xtp.tile([P, C + 1], F32, name="xt", tag="xt")
                for c in range(2):
                    trps = trp.tile([P, P], F32, name="trps", tag="trps")
                    nc.tensor.transpose(
                        trps, xs[c][i][:, j * P:(j + 1) * P], ident_sb
                    )
                    nc.vector.tensor_copy(xt[:, c * P:(c + 1) * P], trps)
                nc.vector.memset(xt[:, C:C + 1], 1.0)
                for b in range(2):
                    nc.tensor.matmul(
                        gx_ps[b],
                        xt[:, b * P:(b + 1) * P],
                        xt,
                        start=(ch == 0),
                        stop=(ch == NCHUNK - 1),
                    )
        for b in range(2):
            nc.vector.tensor_copy(gx_sb[b], gx_ps[b])

    # --- tiny 256x256 algebra -------------------------------------------
    # All matrices in SBUF as two [128, *] row-blocks; vectors as [1, C] rows
    # or [128, 1] per-block columns.
    alg_sb = const  # persistent small tiles live in the const pool

    with tc.tile_pool(name="alg_ps", bufs=3, space="PSUM") as ap:
        # u_row = (w1 sx)^T : lhsT = sx col (gx col 256), rhs = w1t
        u_row = alg_sb.tile([1, C], F32, name="u_row", tag="u_row")
        u_ps = ap.tile([1, C], F32, name="u_ps", tag="alg")
        for k in range(2):
            nc.tensor.matmul(u_ps, gx_sb[k][:, C:C + 1], w1t[k],
                             start=(k == 0), stop=(k == 1))
        nc.vector.tensor_copy(u_row, u_ps)

        # U = (w1 Gx)^T : U[c, o] ; lhsT = Gx[c' blk k, c blk b], rhs = w1t[k]
        u_sb = []
        for b in range(2):
            ups = ap.tile([P, C], F32, name="ups", tag="alg")
            for k in range(2):
                nc.tensor.matmul(ups, gx_sb[k][:, b * P:(b + 1) * P], w1t[k],
                                 start=(k == 0), stop=(k == 1))
            ut = alg_sb.tile([P, C], F32, name=f"u_sb{b}", tag=f"u_sb{b}")
            nc.vector.tensor_copy(ut, ups)
            u_sb.append(ut)

        # G = U^T w1^T (+ rank-1 bias terms); u as column in separate psum
        g_sb = []
        for b in range(2):
            gps = ap.tile([P, C], F32, name="gps", tag="alg")
            ucps = ap.tile([P, 1], F32, name="ucps", tag="algsmall")
            for k in range(2):
                nc.tensor.matmul(gps, u_sb[k][:, b * P:(b + 1) * P],
                                 w1t[k], start=(k == 0), stop=False)
                # u_col block b: lhsT = w1t[k][:, b-slice], rhs = sx col
                nc.tensor.matmul(ucps,
                                 w1t[k][:, b * P:(b + 1) * P],
                                 gx_sb[k][:, C:C + 1],
                                 start=(k == 0), stop=(k == 1))
            nc.tensor.matmul(gps, u_row[:, b * P:(b + 1) * P], b1_row,
                             start=False, stop=False)
            nc.tensor.matmul(gps, b1_row[:, b * P:(b + 1) * P], u_row,
                             start=False, stop=False)
            nc.tensor.matmul(gps, b1_row[:, b * P:(b + 1) * P],
                             nb1_row, start=False, stop=True)
            gt = alg_sb.tile([P, C + 1], F32, name=f"g_sb{b}", tag=f"g_sb{b}")
            nc.vector.tensor_copy(gt[:, 0:C], gps)
            nc.vector.tensor_copy(gt[:, C:C + 1], ucps)
            g_sb.append(gt)

        # s_col = u_col + N*b1 (per block)
        s_col = []
        for k in range(2):
            st = alg_sb.tile([P, 1], F32, name=f"s_col{k}", tag=f"s_col{k}")
            nc.vector.tensor_add(st, g_sb[k][:, C:C + 1], nb1_col[k])
            s_col.append(st)

        # w2s_row = (w2 s)^T, w3s_row = (w3 s)^T
        w2s_row = alg_sb.tile([1, C], F32, name="w2s_row", tag="w2s_row")
        w3s_row = alg_sb.tile([1, C], F32, name="w3s_row", tag="w3s_row")
        for dst, wt in ((w2s_row, w2t), (w3s_row, w3t)):
            vps = ap.tile([1, C], F32, name="vps", tag="alg")
            for k in range(2):
                nc.tensor.matmul(vps, s_col[k], wt[k],
                                 start=(k == 0), stop=(k == 1))
            nc.vector.tensor_copy(dst, vps)

        # U2 = (w2 G)^T
        u2_sb = []
        for b in range(2):
            u2ps = ap.tile([P, C], F32, name="u2ps", tag="alg")
            for k in range(2):
                nc.tensor.matmul(u2ps, g_sb[k][:, b * P:(b + 1) * P], w2t[k],
                                 start=(k == 0), stop=(k == 1))
            u2t = alg_sb.tile([P, C], F32, name=f"u2_sb{b}", tag=f"u2_sb{b}")
            nc.vector.tensor_copy(u2t, u2ps)
            u2_sb.append(u2t)

        # qk = U2^T w3^T + rank-1 terms ; then softmax rows
        attn_sb = []
        for b in range(2):
            qkps = ap.tile([P, C], F32, name="qkps", tag="alg")
            for k in range(2):
                nc.tensor.matmul(qkps, u2_sb[k][:, b * P:(b + 1) * P], w3t[k],
                                 start=(k == 0), stop=False)
            nc.tensor.matmul(qkps, w2s_row[:, b * P:(b + 1) * P], b3_row,
                             start=False, stop=False)
            nc.tensor.matmul(qkps, b2_row[:, b * P:(b + 1) * P], w3s_row,
                             start=False, stop=False)
            nc.tensor.matmul(qkps, b2_row[:, b * P:(b + 1) * P], nb3_row,
                             start=False, stop=True)

            negmax = alg_sb.tile([P, 1], F32, name=f"negmax{b}", tag=f"nm{b}")
            nc.vector.tensor_reduce(
                out=negmax, in_=qkps, op=mybir.AluOpType.max,
                axis=mybir.AxisListType.X, negate=True,
            )
            expq = alg_sb.tile([P, C], F32, name=f"expq{b}", tag=f"expq{b}")
            nc.scalar.activation(
                out=expq, in_=qkps, func=mybir.ActivationFunctionType.Exp,
                bias=negmax, scale=1.0,
            )
            denom = alg_sb.tile([P, 1], F32, name=f"denom{b}", tag=f"dn{b}")
            nc.vector.reduce_sum(out=denom, in_=expq,
                                 axis=mybir.AxisListType.X)
            rden = alg_sb.tile([P, 1], F32, name=f"rden{b}", tag=f"rd{b}")
            nc.vector.reciprocal(rden, denom)
            at = alg_sb.tile([P, C], F32, name=f"attn{b}", tag=f"attn{b}")
            nc.vector.tensor_scalar_mul(at, expq, rden)
            attn_sb.append(at)

        # attn^T (4 PE transposes)
        attnT_sb = [
            alg_sb.tile([P, C], F32, name=f"attnT{j}", tag=f"attnT{j}")
            for j in range(2)
        ]
        for b in range(2):
            for j in range(2):
                tps = ap.tile([P, P], F32, name="tps", tag="alg")
                nc.tensor.transpose(tps, attn_sb[b][:, j * P:(j + 1) * P],
                                    ident_sb)
                nc.vector.tensor_copy(attnT_sb[j][:, b * P:(b + 1) * P], tps)

        # M^T = w4-as-lhsT @ attn^T
        mt_sb = []
        for b in range(2):
            mps = ap.tile([P, C], F32, name="mps", tag="alg")
            for k in range(2):
                nc.tensor.matmul(mps, w4r[k][:, b * P:(b + 1) * P],
                                 attnT_sb[k], start=(k == 0), stop=(k == 1))
            mt = alg_sb.tile([P, C], F32, name=f"mt_sb{b}", tag=f"mt_sb{b}")
            nc.vector.tensor_copy(mt, mps)
            mt_sb.append(mt)

        # W^T = w1-as-lhsT @ M^T
        wt_sb = []
        for b in range(2):
            wps = ap.tile([P, C], F32, name="wps", tag="alg")
            for k in range(2):
                nc.tensor.matmul(wps, w1r[k][:, b * P:(b + 1) * P], mt_sb[k],
                                 start=(k == 0), stop=(k == 1))
            wt_ = alg_sb.tile([P, C], F32, name=f"wt_sb{b}", tag=f"wt_sb{b}")
            nc.vector.tensor_copy(wt_, wps)
            wt_sb.append(wt_)

        # c0_col = M b1 + attn b4 (per block)
        c0_col = []
        for b in range(2):
            cps = ap.tile([P, 1], F32, name="cps", tag="alg")
            for k in range(2):
                nc.tensor.matmul(cps, mt_sb[k][:, b * P:(b + 1) * P],
                                 b1_col[k], start=(k == 0), stop=False)
            for k in range(2):
                nc.tensor.matmul(cps, attnT_sb[k][:, b * P:(b + 1) * P],
                                 b4_col[k], start=False, stop=(k == 1))
            ct = alg_sb.tile([P, 1], F32, name=f"c0_col{b}", tag=f"c0_col{b}")
            nc.vector.tensor_copy(ct, cps)
            c0_col.append(ct)

    # --- pass 2: out = W x + c0 1^T -------------------------------------
    with tc.tile_pool(name="o_ps", bufs=4, space="PSUM") as ops, \
         tc.tile_pool(name="o_sb", bufs=3) as osb:
        nsub = OUTW // NT  # psum tiles per staging tile
        for b in range(2):
            for i in range(NPIX // OUTW):
                ot = osb.tile([P, OUTW], F32, name="ot", tag="ot")
                pst = [
                    ops.tile([P, NT], F32, name="pst", tag="pst")
                    for _ in range(nsub)
                ]
                for k in range(2):
                    for t in range(nsub):
                        col = i * OUTW + t * NT
                        pc, off = divmod(col, PIECE)
                        nc.tensor.matmul(
                            pst[t],
                            wt_sb[k][:, b * P:(b + 1) * P],
                            xs[k][pc][:, off:off + NT],
                            start=(k == 0),
                            stop=(k == 1),
                        )
                for t in range(nsub):
                    nc.vector.tensor_scalar_add(
                        ot[:, t * NT:(t + 1) * NT], pst[t], c0_col[b]
                    )
                nc.sync.dma_start(
                    out=out_d[b * P:(b + 1) * P, i * OUTW:(i + 1) * OUTW],
                    in_=ot,
                )


def build_program(enable_asserts=False):
    nc = bacc.Bacc(
        "TRN2",
        target_bir_lowering=False,
        debug=False,
        enable_asserts=enable_asserts,
        num_devices=8,
    )
    d_in = {
        "xa": nc.dram_tensor("xa", [P, NPIX], F32, kind="ExternalInput").ap(),
        "xb": nc.dram_tensor("xb", [P, NPIX], F32, kind="ExternalInput").ap(),
        "wcat": nc.dram_tensor("wcat", [C, 5 * C], F32,
                               kind="ExternalInput").ap(),
        "brows": nc.dram_tensor("brows", [5, C], F32,
                                kind="ExternalInput").ap(),
        "bcols": nc.dram_tensor("bcols", [C, 3], F32,
                                kind="ExternalInput").ap(),
        "ident": nc.dram_tensor("ident", [P, P], F32,
                                kind="ExternalInput").ap(),
    }
    d_out = {
        "out": nc.dram_tensor("out", [C, NPIX], F32,
                              kind="ExternalOutput").ap(),
    }
    with tile.TileContext(nc) as tc, ExitStack() as ctx:
        _emit(nc, tc, ctx, d_in, d_out)
    nc.compile()
    return nc


def make_in_maps(a, b, w1, b1, w2, b2, w3, b3, w4, b4):
    N = NPIX
    f = np.float32
    wcat = np.concatenate(
        [w1.T, w1, w2.T, w3.T, w4], axis=1
    ).astype(f, copy=False)
    brows = np.stack([b1, N * b1, b2, b3, N * b3]).astype(f, copy=False)
    bcols = np.stack([b1, N * b1, b4], axis=1).astype(f, copy=False)
    ident = np.eye(P, dtype=f)
    B = a.shape[0]
    in_maps = []
    for i in range(B):
        in_maps.append({
            "xa": np.ascontiguousarray(a[i].reshape(P, N)),
            "xb": np.ascontiguousarray(b[i].reshape(P, N)),
            "wcat": wcat,
            "brows": brows,
            "bcols": bcols,
            "ident": ident,
        })
    return in_maps


_CACHE = {}


def kernel(a, b, w1, b1, w2, b2, w3, b3, w4, b4, _trace=False):
    a = np.asarray(a, dtype=np.float32)
    b = np.asarray(b, dtype=np.float32)
    args = [np.asarray(t, dtype=np.float32)
            for t in (w1, b1, w2, b2, w3, b3, w4, b4)]
    if "nc" not in _CACHE:
        _CACHE["nc"] = build_program()
    nc = _CACHE["nc"]
    in_maps = make_in_maps(a, b, *args)
    res = run_bass_kernel_spmd(nc, in_maps, core_ids=list(range(8)),
                               trace=_trace)
    B, Ch, H, W = a.shape
    out = np.stack([r["out"].reshape(C, H, W) for r in res.results])
    if _trace:
        _CACHE["last_results"] = res
    return out
